# revision 27
# baseline (speedup 1.0000x reference)
"""DynamicGraphEmbedding kernel for 8 Trainium2 NeuronCores.

The reference collapses algebraically:
  - deg[i] == K == 16 for every node (dst list is repeat(arange(N), K)),
    so gcn_norm edge weight ew == 1/16 for every edge.
  - straight-through gumbel gate is exactly y_hard in the forward pass,
    i.e. gate(e) = 1 iff argmax(softmax(logits[e] + g[e])) == 0.
  - therefore out[b] = A @ (x[b] @ W) + bias, with the dense [N, N] matrix
    A[i, j] = gate(i*N+j)/16 if j in topk_j[i] else 0.

Host (tiny, O(N^2)): build A from emb/logits/gumbel_u with the exact same
jax-on-CPU ops as the reference. Device (the memory-bound bulk): two chained
256^3 matmuls per batch element, data-parallel over batch across 8 cores.

All device traffic and matmuls are fp16 (halves HBM bytes vs f32 and enables
the PE fast-weight-load path); PSUM accumulation stays f32, output is
upcast to f32 on the host. A entries are gate/16, exact in fp16, so the only
quantization is x/W/h/out rounding: ~1e-3 relative, well under tolerance.

Schedule notes (trace-driven):
  - The NRT preamble pins every kernel instruction after ~7.0us and the
    post-receipt teardown costs a fixed ~2.2us. DMA completion receipts
    process in one global FIFO in issue order: the first lands
    ~2.2us after its transfer, later ones at +0.4..1.4us cadence.
    SWDGE receipts are ~2us slower still -> all loads ride the two
    HWDGE rings (SP, ACT), and fewer DMAs = earlier receipts + less
    run-to-run jitter.
  - One big SP head DMA carries W + A + x pair-0, so a single receipt
    (~11.3us median, ~13.1us under cross-core contention) unlocks both
    the first h-stage AND the first po-stage; pair 1 rides ACT and
    pairs 2+3 ride one second SP DMA, each arriving >=0.6us before PE
    needs it.
  - PE runs at ~1.2 GHz until ~5us after its first activity, and any
    idle gap >~1us re-throttles it (~3us penalty). 44 memset-fed
    128-free warm-up matmuls keep PE busy from ~7.4us through the
    jitter band of the head receipt.
  - po matmuls are split per batch element (bi) so the first po group
    starts right after the DVE half of the h copy, not the full pair;
    the final pair runs m-bank-outer so bank m0's copy+store overlap
    the m1 matmuls.
  - PSUM->SBUF copies (DVE/ACT only: gpsimd cannot read PSUM, and a
    partial-bank read serializes against the whole bank): DVE takes h
    bi=0, ACT takes h bi=1; out-copies are placed so the drain-critical
    pair-3 banks find both engines free; final stores split across both
    HWDGE rings.
"""

import sys

import numpy as np

if "/opt/trn_rl_repo" not in sys.path:
    sys.path.insert(0, "/opt/trn_rl_repo")

N, T, B, D, K = 256, 256, 64, 64, 16
NCORES = 8
BPC = B // NCORES  # batch elements per core
NG = BPC // 2  # batch pairs per core
# 44 x 107ns covers PE from ~7.4us to ~12.1us: the head receipt medians
# ~11.3us but jitters to ~13.1us under cross-core contention, and a PE
# idle gap >~1us re-throttles the clock for ~3us -- tail insurance is
# worth ~0.2us of median.
N_WARM = 44

_CACHE = {}
LAST_RESULT = None  # BassKernelResults of the most recent run (for profiling)


def _graph_matrix(emb, logits, gumbel_u):
    """Dense [N, N] combined gate/topk/gcn-norm matrix A (host-side, tiny)."""
    try:
        import jax
        import jax.numpy as jnp

        cpu = jax.devices("cpu")[0]
        emb_j = jax.device_put(np.asarray(emb), cpu)
        logits_j = jax.device_put(np.asarray(logits), cpu)
        gu_j = jax.device_put(np.asarray(gumbel_u), cpu)
        nrm = jnp.linalg.norm(emb_j, axis=-1)
        cos = (emb_j @ emb_j.T) / (nrm[:, None] * nrm[None, :])
        _, topk_j = jax.lax.top_k(cos, K)
        g = -jnp.log(-jnp.log(gu_j))
        y_soft = jax.nn.softmax(logits_j + g, axis=-1)
        am = jnp.argmax(y_soft, axis=-1)
        topk = np.asarray(topk_j)
        gate_full = (np.asarray(am) == 0).astype(np.float32)
    except Exception:
        emb32 = np.asarray(emb, np.float32)
        nrm = np.sqrt((emb32 * emb32).sum(-1))
        cos = (emb32 @ emb32.T) / (nrm[:, None] * nrm[None, :])
        topk = np.argsort(-cos, axis=-1, kind="stable")[:, :K]
        lg = np.asarray(logits, np.float32) + np.float32(-1.0) * np.log(
            -np.log(np.asarray(gumbel_u, np.float32))
        )
        e = np.exp(lg - lg.max(-1, keepdims=True))
        y_soft = e / e.sum(-1, keepdims=True)
        gate_full = (np.argmax(y_soft, -1) == 0).astype(np.float32)
    rows = np.repeat(np.arange(N), K)
    cols = topk.reshape(-1)
    A = np.zeros((N, N), np.float32)
    A[rows, cols] = gate_full[rows * N + cols] * np.float32(0.0625)
    return A


def _build_bass(with_bias):
    """Per-core Bass graph: out[b] = A @ (x[b] @ W) [+ bias] for BPC batches.

    Host-packed fp16 layouts (contiguous per-partition runs, few big DMAs):
      sp1   [128, 8, 256]         blocks: [W0, W1, A0, A1, x-p0(c,bi)x4]
                                  W c:  [p, c, t']     = W[c*128+p, t']
                                  A c:  [p, 2+c, i]    = A[i, c*128+p]
                                  x:    [p, 4+2c+bi,n] = x[bi][n, c*128+p]
      xin1  [128, 2, 2, 256]      pair 1: [p, c, bi, n]
      xin23 [128, 2, 2, 2, 256]   pairs 2+3 in one DMA: [p, g-2, c, bi, n]
      bias  [1, 256]              (only when with_bias)
      outp  [NG, 128, 2, 2, 256]  [g, p, m, bi, t] = out[2g+bi][m*128+p, t]
    """
    import concourse.bass as bass
    import concourse.mybir as mybir
    from concourse import bacc
    from concourse.tile import TileContext

    F32 = mybir.dt.float32
    F16 = mybir.dt.float16

    nc = bacc.Bacc()
    sp1 = nc.declare_dram_parameter("sp1", [128, 8, 256], F16, isOutput=False)
    xin1 = nc.declare_dram_parameter("xin1", [128, 2, 2, N], F16, isOutput=False)
    xin23 = nc.declare_dram_parameter(
        "xin23", [128, 2, 2, 2, N], F16, isOutput=False
    )
    if with_bias:
        bp = nc.declare_dram_parameter("bias", [1, T], F32, isOutput=False)
    outp = nc.declare_dram_parameter("outp", [NG, 128, 2, 2, T], F16, isOutput=True)

    with TileContext(nc) as tc:
        with (
            tc.tile_pool(name="const", bufs=1) as const,
            tc.tile_pool(name="xpool", bufs=3) as xpool,
            tc.tile_pool(name="hbuf", bufs=3) as hbuf,
            tc.tile_pool(name="obuf", bufs=4) as obuf,
            tc.tile_pool(name="psA", bufs=4, space="PSUM") as psA,
            tc.tile_pool(name="psB", bufs=4, space="PSUM") as psB,
        ):
            sp1t = const.tile([128, 8, 256], F16)
            scratch = const.tile([128, 128], F16, tag="warm")
            nc.gpsimd.memset(scratch, 0.0)
            # One SP DMA delivers W + A + x pair-0: a single completion
            # receipt unlocks the whole h0+po0 prefix of the PE stream.
            nc.sync.dma_start(out=sp1t, in_=sp1.ap())
            # Completion receipts process in one global FIFO (issue order)
            # at ~1.4us cadence, so fewer DMAs means earlier receipts all
            # the way down: p1 rides ACT, pairs 2+3 ride ONE second SP DMA
            # (receipt ~14.4, needed 15.0/16.9). SWDGE receipts are ~2us
            # slower -> unused for loads.
            xt1 = xpool.tile([128, 2, 2, N], F16, tag="xt1")  # [p,c,bi,n]
            nc.scalar.dma_start(out=xt1, in_=xin1.ap())
            x23 = xpool.tile([128, 2, 2, 2, N], F16, tag="x23")  # [p,g,c,bi,n]
            nc.sync.dma_start(out=x23, in_=xin23.ap())
            if with_bias:
                bias_bc = const.tile([128, T], F32)
                nc.gpsimd.dma_start(out=bias_bc, in_=bp.ap().to_broadcast([128, T]))

            # HAM warm-up: keep PE continuously busy from ~7.4us until the
            # head receipt (~11.3us) so real matmuls ramp to 2.4 GHz. An
            # idle PE gap >~1us re-throttles the clock (costs ~3.5us of
            # re-ramp), so the warm-up tail must reach the head receipt.
            # Small [128,128] scratch memsets in ~130ns (vs 520 for 512
            # cols), and 128-free warm-ups give fine-grained tail
            # alignment: ~36 x 107ns at the 1.2 GHz pre-ramp clock.
            wps = psB.tile([128, 2, T], F32, tag="po")
            for _ in range(N_WARM):
                nc.tensor.matmul(
                    wps[:, 0, 0:128],
                    lhsT=scratch,
                    rhs=scratch,
                    start=True,
                    stop=True,
                )

            def wmat(c):
                return sp1t[:, c]  # [128, 256] W rows c*128+p

            def amat(c):
                return sp1t[:, 2 + c]  # [128, 256] A^T rows c*128+p

            def xap(g, c, bi, mslice):
                """lhsT chunk [128, 128] for pair g, contraction chunk c."""
                if g == 0:
                    return sp1t[:, 4 + 2 * c + bi, mslice]
                if g == 1:
                    return xt1[:, c, bi, mslice]
                return x23[:, g - 2, c, bi, mslice]

            h_sbs = {}

            def emit_h(g):
                # h for the pair: [p=j%128, bi, jc(=node block m), t']
                h_sb = hbuf.tile([128, 2, 2, T], F16, tag="h_sb")
                h_sbs[g] = h_sb
                for bi in range(2):
                    # both m chunks share one PSUM bank -> single wide copy
                    ph = psA.tile([128, 2, T], F32, tag="ph")
                    for m in range(2):
                        nc.tensor.matmul(
                            ph[:, m],
                            lhsT=xap(g, 0, bi, bass.ts(m, 128)),
                            rhs=wmat(0),
                            start=True,
                            stop=False,
                        )
                        nc.tensor.matmul(
                            ph[:, m],
                            lhsT=xap(g, 1, bi, bass.ts(m, 128)),
                            rhs=wmat(1),
                            start=False,
                            stop=True,
                        )
                    # PSUM->SBUF cast copy, [128, 512] contiguous. (Reading
                    # one half of the bank while PE still streams the other
                    # half races on HW -- copy only after both groups stop.)
                    # bi=0 rides DVE, bi=1 rides ACT: the first po group of
                    # the pair only needs the DVE half.
                    if bi == 0:
                        nc.vector.tensor_copy(h_sb[:, 0], ph)
                    else:
                        nc.scalar.copy(out=h_sb[:, 1], in_=ph)

            def emit_po(g):
                h_sb = h_sbs[g]
                ob = obuf.tile([128, 2, 2, T], F16, tag="ob")  # [p, m, bi, t]
                po0 = psB.tile([128, 2, T], F32, tag="po")
                po1 = psB.tile([128, 2, T], F32, tag="po")
                pos = [po0, po1]
                # bi-split matmuls: the bi=0 groups depend only on the DVE
                # half-copy, so po starts ~0.5us before the ACT half lands.
                # The final pair runs m-bank-outer instead so bank m0 stops
                # ~0.45us before the last matmul and its copy/store overlap
                # the m1 matmuls (drain-critical).
                loop = (
                    [(m, bi) for m in range(2) for bi in range(2)]
                    if g == NG - 1
                    else [(m, bi) for bi in range(2) for m in range(2)]
                )
                for m, bi in loop:
                    nc.tensor.matmul(
                        pos[m][:, bi],
                        lhsT=amat(0)[:, bass.ts(m, 128)],
                        rhs=h_sb[:, bi, 0, :],
                        start=True,
                        stop=False,
                    )
                    nc.tensor.matmul(
                        pos[m][:, bi],
                        lhsT=amat(1)[:, bass.ts(m, 128)],
                        rhs=h_sb[:, bi, 1, :],
                        start=False,
                        stop=True,
                    )

                def ocopy(eng, m):
                    if with_bias:
                        # only DVE can both read PSUM and tensor_add
                        for bi in range(2):
                            nc.vector.tensor_add(
                                ob[:, m, bi, :], pos[m][:, bi, :], bias_bc
                            )
                    elif eng is nc.scalar:
                        eng.copy(out=ob[:, m], in_=pos[m])
                    else:
                        eng.tensor_copy(ob[:, m], pos[m])

                # Copy-engine split (gpsimd cannot read PSUM; a partial-
                # bank PSUM read serializes against the WHOLE bank, so
                # copies are always full-bank): pair 2's m0 bank stops
                # while DVE is already done with h3-bi0, so DVE takes it;
                # ACT takes m1 after its h3-bi1 copy. For the final pair
                # the m-outer matmul order lets ACT copy+store m0 while PE
                # still streams m1.
                if g == 2:
                    ocopy(nc.vector, 0)
                    ocopy(nc.scalar, 1)
                else:
                    ocopy(nc.scalar, 0)
                    ocopy(nc.vector, 1)
                # Stores alternate rings; the final pair splits m0/m1 across
                # both rings so the two last transfers run in parallel.
                if g == 0:
                    nc.sync.dma_start(out=outp[0], in_=ob)
                elif g == 1:
                    nc.scalar.dma_start(out=outp[1], in_=ob)
                elif g == 2:
                    nc.sync.dma_start(out=outp[2], in_=ob)
                else:
                    nc.scalar.dma_start(out=outp[3][:, 0], in_=ob[:, 0])
                    nc.sync.dma_start(out=outp[3][:, 1], in_=ob[:, 1])

            # po1 waits for pair-1 copies -> slot h2 before it so PE never
            # idles; the four ph PSUM banks exactly cover two pairs in
            # flight (h2 reuses h0's banks after their copies complete).
            emit_h(0)
            emit_po(0)
            emit_h(1)
            emit_h(2)
            emit_po(1)
            emit_h(3)
            emit_po(2)
            emit_po(3)
    nc.finalize()
    return nc


def _ensure_axon_hooks_importable():
    """concourse's trace path hard-imports antenv.axon_hooks, which this
    image lacks. Provide the real ctypes-backed hook when possible, else a
    no-op, so BASS_TRACE=1 degrades gracefully instead of crashing."""
    try:
        import antenv.axon_hooks  # noqa: F401

        return
    except ImportError:
        pass
    try:
        import types

        import antenv

        mod = types.ModuleType("antenv.axon_hooks")
        state = {"h": None}
        mod.set_axon_ntff_profile_hook = lambda h: state.__setitem__("h", h)
        mod.get_axon_ntff_profile_hook = lambda: state["h"]
        sys.modules["antenv.axon_hooks"] = mod
        antenv.axon_hooks = mod
        try:
            from trn_agent_boot.trn_boot import _ntff_profile_via_ctypes

            hook = _ntff_profile_via_ctypes("/opt/axon/libaxon_pjrt.so")
            if hook is not None:
                mod.set_axon_ntff_profile_hook(hook)
        except Exception:
            pass
    except Exception:
        pass


def kernel(x, emb, W, b, logits, gumbel_u):
    global LAST_RESULT
    _ensure_axon_hooks_importable()
    from concourse.bass_utils import run_bass_kernel_spmd

    x = np.asarray(x, np.float32)
    W = np.asarray(W, np.float32)
    bias = np.ascontiguousarray(np.asarray(b, np.float32)).reshape(1, T)

    A = _graph_matrix(emb, logits, gumbel_u)
    W16 = W.astype(np.float16)  # [t, t'], t = c*128 + p
    A16 = np.ascontiguousarray(A.T).astype(np.float16)  # [j, i], j = c*128 + p
    apack = np.ascontiguousarray(A16.reshape(2, 128, N).transpose(1, 0, 2))

    # xin [B/2 pairs, p, c, bi, n]: xT[b][t, n] split t = c*128+p, b = 2g+bi
    xT = x.transpose(0, 2, 1).astype(np.float16)  # [B, T, N]
    xpack = np.ascontiguousarray(
        xT.reshape(B // 2, 2, 2, 128, N).transpose(0, 3, 2, 1, 4)
    )  # [g, p, c, bi, n]

    with_bias = bool(np.any(bias))
    key = ("nc", with_bias)
    if key not in _CACHE:
        _CACHE[key] = _build_bass(with_bias)
    nc = _CACHE[key]

    # sp1 [128, 8, 256] per core: [W0, W1, A0, A1, x pair-0 (c,bi)-major]
    wr = W16.reshape(2, 128, T)  # [c, p, t']
    in_maps = []
    for c in range(NCORES):
        xg = xpack[c * NG : (c + 1) * NG]  # [NG, p, c, bi, n]
        head = np.concatenate(
            [
                wr[0][:, None, :],
                wr[1][:, None, :],
                apack[:, 0:1, :],
                apack[:, 1:2, :],
                xg[0].reshape(128, 4, N),
            ],
            axis=1,
        )
        in_maps.append(
            {
                "sp1": np.ascontiguousarray(head),
                "xin1": np.ascontiguousarray(xg[1]),
                "xin23": np.ascontiguousarray(
                    np.stack([xg[2], xg[3]], axis=1)
                ),  # [p, g-2, c, bi, n]
            }
        )
    if with_bias:
        for m in in_maps:
            m["bias"] = bias
    # The first execution of a fresh NEFF occasionally trips a transient
    # NRT_EXEC_UNIT_UNRECOVERABLE; a straight retry reliably succeeds. A
    # crashed attempt can leave the NTFF profiler session open, which makes
    # a traced retry die at axon_start_nrt_profile -- so retries run with
    # tracing disabled (correctness over profiling).
    import os as _os

    last_exc = None
    had_never_trace = "BASS_NEVER_TRACE" in _os.environ
    try:
        for _attempt in range(4):
            try:
                res = run_bass_kernel_spmd(
                    nc, in_maps, core_ids=list(range(NCORES))
                )
                break
            except Exception as e:  # noqa: BLE001
                last_exc = e
                import time as _time

                _time.sleep(2.0)
                _os.environ["BASS_NEVER_TRACE"] = "1"
        else:
            raise last_exc
    finally:
        if not had_never_trace:
            _os.environ.pop("BASS_NEVER_TRACE", None)
    LAST_RESULT = res
    # outp [NG, p, m, bi, t] -> out[2g+bi, m*128+p, t], upcast to f32
    out = np.empty((B, N, T), np.float32)
    for c in range(NCORES):
        ob = res.results[c]["outp"].astype(np.float32)  # [NG,128,2,2,T]
        ob = ob.transpose(0, 3, 2, 1, 4).reshape(BPC, N, T)  # [2g+bi, m*128+p, t]
        out[c * BPC : (c + 1) * BPC] = ob
    return out
